# revision 13
# baseline (speedup 1.0000x reference)
"""Trainium2 Bass kernel for ClassicalSelfAttention.

  out = softmax((x @ Wq) @ (x @ Wk)^T / sqrt(D)) @ x      x: [8192, 1024] f32

Sharding (8 NeuronCores): rows of x are sharded across cores; each core
projects its own row-shard to Q^T and K^T, the K^T shards are AllGathered
across cores, and each core runs a streaming attention loop over 16
key-blocks of 512 keys: scores matmul -> fused exp on ScalarE -> PV matmul
accumulated in SBUF.  The scores matmul keeps K^T stationary and Q^T
moving, so PSUM holds scores TRANSPOSED ([key, query]); exp of that is
P^T directly, which is what the PV matmul consumes.

The kernel is power-limited (PE drops to ~2.0 GHz under sustained bf16
matmul load), so the main lever is fewer matmul instructions:
 - 6/8 of the 1024-dim score contraction runs in fp8e4 DoubleRow mode
   (2 contraction chunks per instruction): 8-MM chains -> 3 DR + 2 bf16.
   Sim-validated rel err ~1.83e-2 < 2e-2 (sim matched HW to ~1e-5 at
   the 4/8 setting).
 - Softmax denominators are a ones-vector matmul partition reduction
   chained across each block pair.
 - PV is computed transposed (out^T = V^T P^T with V chunks stationary):
   the final divide is an elementwise multiply by a replicated
   reciprocal row (no PE transposes); the host transposes the result.
 - Key blocks are processed in PAIRS: PV / sums accumulation chains span
   both blocks of a pair, halving PSUM->SBUF drain traffic on VectorE.
 - K^T AllGather is split into 4 collectives (fp8/bf16 x key-half) and
   remote blocks are processed half-0-first, so the gather is hidden
   behind own-block compute.
 - 1/sqrt(D) is folded as sqrt(1/32) into BOTH Wq and Wk on the host so
   Q,K land in fp8-friendly range (std ~0.11).
"""

import sys

import numpy as np

try:
    import concourse.bass as bass  # noqa: F401
except ImportError:  # pragma: no cover
    sys.path.insert(0, "/opt/trn_rl_repo")

import concourse.bacc as bacc
import concourse.mybir as mybir
import concourse.tile as tile
from concourse import bass_utils
from concourse.bass import ds

N_TOKENS = 8192
EMBED = 1024
NCORES = 8
M = N_TOKENS // NCORES  # rows per core (1024)
P = 128  # partitions
DC = EMBED // P  # contraction chunks (8)
NB = 512  # key-block width
VC = NB // P  # 128-wide key chunks per key block (4)
HPR = M // NB  # key-block halves per rank (2)

F8C = 6  # contraction chunks (of 8) done in fp8 DoubleRow
BFC = DC - F8C  # contraction chunks done in bf16

FP32 = mybir.dt.float32
BF16 = mybir.dt.bfloat16
F8E4 = mybir.dt.float8e4
EXP = mybir.ActivationFunctionType.Exp
ADD = mybir.AluOpType.add
MULT = mybir.AluOpType.mult
DR = mybir.MatmulPerfMode.DoubleRow

# key-block pairs (j, half): own pair first; then remote, half-0 first
# so the split AllGather pipelines; pair partners share PV/sums chains.
PAIRS = [((0, 0), (0, 1)), ((1, 0), (2, 0)), ((3, 0), (4, 0)),
         ((5, 0), (6, 0)), ((7, 0), (1, 1)), ((2, 1), (3, 1)),
         ((4, 1), (5, 1)), ((6, 1), (7, 1))]


def _build():
    nc = bacc.Bacc(
        "TRN2", target_bir_lowering=False, debug=False, num_devices=NCORES
    )
    xt_shard = nc.dram_tensor("xt_shard", [EMBED, M], BF16, kind="ExternalInput").ap()
    x_shard = nc.dram_tensor("x_shard", [M, EMBED], BF16, kind="ExternalInput").ap()
    x_full = nc.dram_tensor(
        "x_full", [N_TOKENS, EMBED], BF16, kind="ExternalInput"
    ).ap()
    wq_d = nc.dram_tensor("wq", [EMBED, EMBED], BF16, kind="ExternalInput").ap()
    wk_d = nc.dram_tensor("wk", [EMBED, EMBED], BF16, kind="ExternalInput").ap()
    # out^T in bf16: [EMBED, M]; the host transposes back and widens.
    out_d = nc.dram_tensor("out", [EMBED, M], BF16, kind="ExternalOutput").ap()

    wq_r = wq_d.rearrange("(a p) d -> a p d", p=P)  # [DC, P, EMBED]
    wk_r = wk_d.rearrange("(a p) d -> a p d", p=P)
    xt_r = xt_shard.rearrange("(a p) m -> a p m", p=P)  # [DC, P, M]
    xs_r = x_shard.rearrange("(t p) d -> t p d", p=P)  # [M//P, P, EMBED]
    xv_r = x_full.rearrange("(t p) d -> t p d", p=P)  # [64, P, EMBED]
    out_r = out_d.rearrange("(e p) m -> e p m", p=P)  # [DC, P, M]

    with tile.TileContext(nc) as tc:
        with (
            tc.tile_pool(name="persist", bufs=1) as pers,
            tc.tile_pool(name="persist_dram", bufs=1, space="DRAM") as pdram,
        ):
            # own Q^T / K^T, split by precision: fp8 chunks 0..F8C-1,
            # bf16 chunks F8C..7.  Layout [P, chunk, M].
            qt8 = pers.tile([P, F8C, M], F8E4)
            qtb = pers.tile([P, BFC, M], BF16)
            kt8 = pers.tile([P, F8C, M], F8E4)
            ktb = pers.tile([P, BFC, M], BF16)
            ones_bf = pers.tile([P, P], BF16)
            nc.vector.memset(ones_bf[:], 1.0)
            # fp32 PV^T accumulator: [p, e*M + m] (e = embed chunk)
            out_acc = pers.tile([P, DC * M], FP32)
            # softmax denominators, replicated across partitions: [p, m]
            sums_acc = pers.tile([P, M], FP32)
            recip = pers.tile([P, M], FP32)
            # K^T shard (AllGather inputs) and gathered K^T of all cores,
            # split by precision and key-half for collective pipelining.
            ktd8 = [pdram.tile([F8C, P, NB], F8E4, name=f"ktd8_{h}") for h in range(2)]
            ktdb = [pdram.tile([BFC, P, NB], BF16, name=f"ktdb_{h}") for h in range(2)]
            gkt8 = [
                pdram.tile([NCORES * F8C, P, NB], F8E4, addr_space="Shared",
                           name=f"gkt8_{h}")
                for h in range(2)
            ]
            gktb = [
                pdram.tile([NCORES * BFC, P, NB], BF16, addr_space="Shared",
                           name=f"gktb_{h}")
                for h in range(2)
            ]

            rank = nc.gpsimd.partition_id()

            # ---- Phase A: project K^T then Q^T; AllGather K^T (split)
            with (
                tc.tile_pool(name="proj", bufs=1) as proj,
                tc.tile_pool(name="proj_ps", bufs=4, space="PSUM") as proj_ps,
            ):
                wq_sb = proj.tile([P, DC * EMBED], BF16)
                wk_sb = proj.tile([P, DC * EMBED], BF16)
                xt_sb = proj.tile([P, DC * M], BF16)
                # wk/xt feed the first (K) projection - issue them
                # first, on separate trigger rings to parallelize
                # descriptor processing during startup.
                for a in range(DC):
                    nc.sync.dma_start(
                        out=wk_sb[:, a * EMBED : (a + 1) * EMBED], in_=wk_r[a]
                    )
                    nc.scalar.dma_start(
                        out=xt_sb[:, a * M : (a + 1) * M], in_=xt_r[a]
                    )
                for a in range(DC):
                    nc.gpsimd.dma_start(
                        out=wq_sb[:, a * EMBED : (a + 1) * EMBED], in_=wq_r[a]
                    )
                # K^T first so its AllGather overlaps the Q^T projection.
                for w_sb, d8, db, is_k in (
                    (wk_sb, kt8, ktb, True),
                    (wq_sb, qt8, qtb, False),
                ):
                    for j in range(HPR):  # row half
                        for b in range(DC):  # output dim chunk
                            ps = proj_ps.tile([P, NB], FP32, tag="proj_ps")
                            for a in range(DC):  # contraction chunk
                                nc.tensor.matmul(
                                    ps[:],
                                    lhsT=w_sb[:, a * EMBED + b * P : a * EMBED + (b + 1) * P],
                                    rhs=xt_sb[:, a * M + j * NB : a * M + (j + 1) * NB],
                                    start=(a == 0),
                                    stop=(a == DC - 1),
                                )
                            if b < F8C:
                                dst = d8[:, b, j * NB : (j + 1) * NB]
                            else:
                                dst = db[:, b - F8C, j * NB : (j + 1) * NB]
                            nc.vector.tensor_copy(out=dst, in_=ps[:])
                            if is_k:
                                if b < F8C:
                                    nc.sync.dma_start(out=ktd8[j][b], in_=dst)
                                else:
                                    nc.sync.dma_start(out=ktdb[j][b - F8C], in_=dst)
                        if is_k:
                            for td, gd in ((ktdb[j], gktb[j]), (ktd8[j], gkt8[j])):
                                nc.gpsimd.collective_compute(
                                    "AllGather",
                                    mybir.AluOpType.bypass,
                                    replica_groups=[list(range(NCORES))],
                                    ins=[td.opt()],
                                    outs=[gd.opt()],
                                )

            # ---- Phase B: streaming attention over key-block pairs
            with (
                tc.tile_pool(name="kv", bufs=4) as kvp,
                tc.tile_pool(name="pb", bufs=3) as pbp,
                tc.tile_pool(name="ps_s", bufs=1, space="PSUM") as ps_sp,
                tc.tile_pool(name="ps_u", bufs=2, space="PSUM") as ps_up,
                tc.tile_pool(name="ps_o", bufs=2, space="PSUM") as ps_op,
                tc.tile_pool(name="fin", bufs=2) as finp,
            ):
                def load_block(j, half):
                    """Fetch V and K tiles for block (j, half); return
                    (vtile, k8_slice(ap, c), kb_slice(bb, c))."""
                    vtile = kvp.tile([P, VC, EMBED], BF16, tag="vtile")
                    if j == 0:
                        for c in range(VC):
                            nc.sync.dma_start(
                                out=vtile[:, c, :], in_=xs_r[half * VC + c]
                            )

                        def k8s(ap, c, _h=half):
                            return kt8[
                                :, 2 * ap : 2 * ap + 2,
                                _h * NB + c * P : _h * NB + (c + 1) * P,
                            ]

                        def kbs(bb, c, _h=half):
                            return ktb[
                                :, bb, _h * NB + c * P : _h * NB + (c + 1) * P
                            ]

                    else:
                        src = (rank + j) % NCORES
                        for c in range(VC):
                            nc.gpsimd.dma_start(
                                out=vtile[:, c, :],
                                in_=xv_r[
                                    ds(src * (M // P) + half * VC + c, 1)
                                ].squeeze(0),
                            )
                        kt8_t = kvp.tile([P, F8C, NB], F8E4, tag="kt8_t")
                        ktb_t = kvp.tile([P, BFC, NB], BF16, tag="ktb_t")
                        for b in range(F8C):
                            nc.gpsimd.dma_start(
                                out=kt8_t[:, b, :],
                                in_=gkt8[half][ds(src * F8C + b, 1)].squeeze(0),
                            )
                        for b in range(BFC):
                            nc.gpsimd.dma_start(
                                out=ktb_t[:, b, :],
                                in_=gktb[half][ds(src * BFC + b, 1)].squeeze(0),
                            )

                        def k8s(ap, c, _t=kt8_t):
                            return _t[:, 2 * ap : 2 * ap + 2, c * P : (c + 1) * P]

                        def kbs(bb, c, _t=ktb_t):
                            return _t[:, bb, c * P : (c + 1) * P]

                    return vtile, k8s, kbs

                def scores_exp(k8s, kbs):
                    """Score chains + exp for one block; returns the
                    P^T tile (bf16, [P, VC, M]).  The four key-chunk
                    chains of a query-half run as concurrent
                    accumulation groups in four PSUM banks, all bf16
                    matmuls first then all DoubleRow ones, so the PE
                    pays one mode switch per group instead of four."""
                    pt_sb = pbp.tile([P, VC, M], BF16, tag="pt_sb")
                    for h in range(HPR):  # query column half
                        banks = [
                            ps_sp.tile([P, NB], FP32, tag=f"ps_s{c}",
                                       name=f"ps_s{c}")
                            for c in range(VC)
                        ]
                        for bb in range(BFC):
                            for c in range(VC):
                                nc.tensor.matmul(
                                    banks[c][:],
                                    lhsT=kbs(bb, c),
                                    rhs=qtb[:, bb, h * NB : (h + 1) * NB],
                                    start=(bb == 0),
                                    stop=False,
                                )
                        for ap in range(F8C // 2):
                            for c in range(VC):
                                nc.tensor.matmul(
                                    banks[c][:],
                                    lhsT=k8s(ap, c),
                                    rhs=qt8[:, 2 * ap : 2 * ap + 2, h * NB : (h + 1) * NB],
                                    start=False,
                                    stop=(ap == F8C // 2 - 1),
                                    perf_mode=DR,
                                )
                        for c in range(VC):
                            nc.scalar.activation(
                                out=pt_sb[:, c, h * NB : (h + 1) * NB],
                                in_=banks[c][:],
                                func=EXP,
                            )
                    return pt_sb

                def sums_chain(pta, ptb, h, first):
                    # softmax partition-dim sums via ones-vector matmul,
                    # chained across both blocks of the pair.
                    ps_sum = ps_up.tile([P, NB], FP32, tag="ps_sum")
                    for bi, ptx in enumerate((pta, ptb)):
                        for c in range(VC):
                            nc.tensor.matmul(
                                ps_sum[:],
                                lhsT=ones_bf[:],
                                rhs=ptx[:, c, h * NB : (h + 1) * NB],
                                start=(bi == 0 and c == 0),
                                stop=(bi == 1 and c == VC - 1),
                            )
                    dsts = sums_acc[:, h * NB : (h + 1) * NB]
                    if first:
                        nc.vector.tensor_copy(out=dsts, in_=ps_sum[:])
                    else:
                        nc.vector.tensor_tensor(
                            out=dsts, in0=dsts, in1=ps_sum[:], op=ADD
                        )

                def pv_chains(va, vb, pta, ptb, h, first):
                    # PV^T: out^T[e, q] += V[k, e]^T @ P^T[k, q],
                    # chained across both blocks of the pair.
                    for e in range(DC):
                        ps_o = ps_op.tile([P, NB], FP32, tag="ps_o")
                        for vx, ptx in ((va, pta), (vb, ptb)):
                            for t in range(VC):
                                nc.tensor.matmul(
                                    ps_o[:],
                                    lhsT=vx[:, t, e * P : (e + 1) * P],
                                    rhs=ptx[:, t, h * NB : (h + 1) * NB],
                                    start=(vx is va and t == 0),
                                    stop=(vx is vb and t == VC - 1),
                                )
                        dst = out_acc[:, e * M + h * NB : e * M + (h + 1) * NB]
                        if first:
                            nc.vector.tensor_copy(out=dst, in_=ps_o[:])
                        else:
                            nc.vector.tensor_tensor(
                                out=dst, in0=dst, in1=ps_o[:], op=ADD
                            )

                def finalize(h):
                    # out_acc columns for query-half h are final: scale
                    # by 1/sums and ship out (overlaps remaining PV).
                    for e in range(DC):
                        outf = finp.tile([P, NB], BF16, tag="outf")
                        nc.vector.tensor_tensor(
                            out=outf[:],
                            in0=out_acc[:, e * M + h * NB : e * M + (h + 1) * NB],
                            in1=recip[:, h * NB : (h + 1) * NB],
                            op=MULT,
                        )
                        nc.sync.dma_start(
                            out=out_r[e][:, h * NB : (h + 1) * NB], in_=outf[:]
                        )

                for pi, (blk_a, blk_b) in enumerate(PAIRS):
                    first = pi == 0
                    last = pi == len(PAIRS) - 1
                    va, k8a, kba = load_block(*blk_a)
                    vb, k8b, kbb = load_block(*blk_b)
                    pta = scores_exp(k8a, kba)
                    ptb = scores_exp(k8b, kbb)
                    if not last:
                        for h in range(HPR):
                            sums_chain(pta, ptb, h, first)
                            pv_chains(va, vb, pta, ptb, h, first)
                    else:
                        # tail-hiding order: finish sums first so the
                        # reciprocal and the h=0 output scaling overlap
                        # the remaining PV matmul work.
                        for h in range(HPR):
                            sums_chain(pta, ptb, h, first)
                        nc.vector.reciprocal(out=recip[:], in_=sums_acc[:])
                        pv_chains(va, vb, pta, ptb, 0, first)
                        finalize(0)
                        pv_chains(va, vb, pta, ptb, 1, first)
                        finalize(1)

    nc.compile()
    return nc


_NC = None


def _get_nc():
    global _NC
    if _NC is None:
        _NC = _build()
    return _NC


def _run(x, rotation_params, entangle_params, **spmd_kwargs):
    import ml_dtypes

    x = np.ascontiguousarray(np.asarray(x, dtype=np.float32))
    sc = np.float32(1.0 / np.sqrt(np.sqrt(np.float32(EMBED))))
    wq = (np.asarray(rotation_params, dtype=np.float32).reshape(EMBED, EMBED) * sc).astype(
        ml_dtypes.bfloat16
    )
    wk = (np.asarray(entangle_params, dtype=np.float32).reshape(EMBED, EMBED) * sc).astype(
        ml_dtypes.bfloat16
    )
    x_bf = x.astype(ml_dtypes.bfloat16)
    xt_bf = np.ascontiguousarray(x_bf.T)
    in_maps = [
        {
            "xt_shard": np.ascontiguousarray(xt_bf[:, i * M : (i + 1) * M]),
            "x_shard": np.ascontiguousarray(x_bf[i * M : (i + 1) * M]),
            "x_full": x_bf,
            "wq": wq,
            "wk": wk,
        }
        for i in range(NCORES)
    ]
    res = bass_utils.run_bass_kernel_spmd(
        _get_nc(), in_maps, core_ids=list(range(NCORES)), **spmd_kwargs
    )
    out = np.concatenate(
        [np.asarray(res.results[i]["out"]).astype(np.float32).T for i in range(NCORES)],
        axis=0,
    )
    return np.ascontiguousarray(out), res


def kernel(x, rotation_params, entangle_params):
    out, _ = _run(x, rotation_params, entangle_params)
    return out


# revision 14
# speedup vs baseline: 1.0008x; 1.0008x over previous
"""Trainium2 Bass kernel for ClassicalSelfAttention.

  out = softmax((x @ Wq) @ (x @ Wk)^T / sqrt(D)) @ x      x: [8192, 1024] f32

Sharding (8 NeuronCores): rows of x are sharded across cores; each core
projects its own row-shard to Q^T and K^T, the K^T shards are AllGathered
across cores, and each core runs a streaming attention loop over 16
key-blocks of 512 keys: scores matmul -> fused exp on ScalarE -> PV matmul
accumulated in SBUF.  The scores matmul keeps K^T stationary and Q^T
moving, so PSUM holds scores TRANSPOSED ([key, query]); exp of that is
P^T directly, which is what the PV matmul consumes.

The kernel is power-limited (PE drops to ~2.0 GHz under sustained bf16
matmul load), so the main lever is fewer matmul instructions:
 - 6/8 of the 1024-dim score contraction runs in fp8e4 DoubleRow mode
   (2 contraction chunks per instruction): 8-MM chains -> 3 DR + 2 bf16.
   Sim-validated rel err ~1.83e-2 < 2e-2 (sim matched HW to ~1e-5 at
   the 4/8 setting).
 - Softmax denominators are a ones-vector matmul partition reduction
   chained across each block pair.
 - PV is computed transposed (out^T = V^T P^T with V chunks stationary):
   the final divide is an elementwise multiply by a replicated
   reciprocal row (no PE transposes); the host transposes the result.
 - Key blocks are processed in PAIRS: PV / sums accumulation chains span
   both blocks of a pair, halving PSUM->SBUF drain traffic on VectorE.
 - K^T AllGather is split into 4 collectives (fp8/bf16 x key-half) and
   remote blocks are processed half-0-first, so the gather is hidden
   behind own-block compute.
 - 1/sqrt(D) is folded as sqrt(1/32) into BOTH Wq and Wk on the host so
   Q,K land in fp8-friendly range (std ~0.11).
"""

import sys

import numpy as np

try:
    import concourse.bass as bass  # noqa: F401
except ImportError:  # pragma: no cover
    sys.path.insert(0, "/opt/trn_rl_repo")

import concourse.bacc as bacc
import concourse.mybir as mybir
import concourse.tile as tile
from concourse import bass_utils
from concourse.bass import ds

N_TOKENS = 8192
EMBED = 1024
NCORES = 8
M = N_TOKENS // NCORES  # rows per core (1024)
P = 128  # partitions
DC = EMBED // P  # contraction chunks (8)
NB = 512  # key-block width
VC = NB // P  # 128-wide key chunks per key block (4)
HPR = M // NB  # key-block halves per rank (2)

F8C = 6  # contraction chunks (of 8) done in fp8 DoubleRow
BFC = DC - F8C  # contraction chunks done in bf16

FP32 = mybir.dt.float32
BF16 = mybir.dt.bfloat16
F8E4 = mybir.dt.float8e4
EXP = mybir.ActivationFunctionType.Exp
ADD = mybir.AluOpType.add
MULT = mybir.AluOpType.mult
DR = mybir.MatmulPerfMode.DoubleRow

# key-block pairs (j, half): own pair first; then remote, half-0 first
# so the split AllGather pipelines; pair partners share PV/sums chains.
PAIRS = [((0, 0), (0, 1)), ((1, 0), (2, 0)), ((3, 0), (4, 0)),
         ((5, 0), (6, 0)), ((7, 0), (1, 1)), ((2, 1), (3, 1)),
         ((4, 1), (5, 1)), ((6, 1), (7, 1))]


def _build():
    nc = bacc.Bacc(
        "TRN2", target_bir_lowering=False, debug=False, num_devices=NCORES
    )
    xt_shard = nc.dram_tensor("xt_shard", [EMBED, M], BF16, kind="ExternalInput").ap()
    x_shard = nc.dram_tensor("x_shard", [M, EMBED], BF16, kind="ExternalInput").ap()
    x_full = nc.dram_tensor(
        "x_full", [N_TOKENS, EMBED], BF16, kind="ExternalInput"
    ).ap()
    wq_d = nc.dram_tensor("wq", [EMBED, EMBED], BF16, kind="ExternalInput").ap()
    wk_d = nc.dram_tensor("wk", [EMBED, EMBED], BF16, kind="ExternalInput").ap()
    # out^T in bf16: [EMBED, M]; the host transposes back and widens.
    out_d = nc.dram_tensor("out", [EMBED, M], BF16, kind="ExternalOutput").ap()

    wq_r = wq_d.rearrange("(a p) d -> a p d", p=P)  # [DC, P, EMBED]
    wk_r = wk_d.rearrange("(a p) d -> a p d", p=P)
    xt_r = xt_shard.rearrange("(a p) m -> a p m", p=P)  # [DC, P, M]
    xs_r = x_shard.rearrange("(t p) d -> t p d", p=P)  # [M//P, P, EMBED]
    xv_r = x_full.rearrange("(t p) d -> t p d", p=P)  # [64, P, EMBED]
    out_r = out_d.rearrange("(e p) m -> e p m", p=P)  # [DC, P, M]

    with tile.TileContext(nc) as tc:
        with (
            tc.tile_pool(name="persist", bufs=1) as pers,
            tc.tile_pool(name="persist_dram", bufs=1, space="DRAM") as pdram,
        ):
            # own Q^T / K^T, split by precision: fp8 chunks 0..F8C-1,
            # bf16 chunks F8C..7.  Layout [P, chunk, M].
            qt8 = pers.tile([P, F8C, M], F8E4)
            qtb = pers.tile([P, BFC, M], BF16)
            kt8 = pers.tile([P, F8C, M], F8E4)
            ktb = pers.tile([P, BFC, M], BF16)
            ones_bf = pers.tile([P, P], BF16)
            nc.vector.memset(ones_bf[:], 1.0)
            # fp32 PV^T accumulator: [p, e*M + m] (e = embed chunk)
            out_acc = pers.tile([P, DC * M], FP32)
            # softmax denominators, replicated across partitions: [p, m]
            sums_acc = pers.tile([P, M], FP32)
            recip = pers.tile([P, M], FP32)
            # K^T shard (AllGather inputs) and gathered K^T of all cores,
            # split by precision and key-half for collective pipelining.
            ktd8 = [pdram.tile([F8C, P, NB], F8E4, name=f"ktd8_{h}") for h in range(2)]
            ktdb = [pdram.tile([BFC, P, NB], BF16, name=f"ktdb_{h}") for h in range(2)]
            gkt8 = [
                pdram.tile([NCORES * F8C, P, NB], F8E4, addr_space="Shared",
                           name=f"gkt8_{h}")
                for h in range(2)
            ]
            gktb = [
                pdram.tile([NCORES * BFC, P, NB], BF16, addr_space="Shared",
                           name=f"gktb_{h}")
                for h in range(2)
            ]

            rank = nc.gpsimd.partition_id()

            # ---- Phase A: project K^T then Q^T; AllGather K^T (split)
            with (
                tc.tile_pool(name="proj", bufs=1) as proj,
                tc.tile_pool(name="proj_ps", bufs=4, space="PSUM") as proj_ps,
            ):
                wq_sb = proj.tile([P, DC * EMBED], BF16)
                wk_sb = proj.tile([P, DC * EMBED], BF16)
                xt_sb = proj.tile([P, DC * M], BF16)
                # wk/xt feed the first (K) projection - issue them first.
                for a in range(DC):
                    nc.sync.dma_start(
                        out=wk_sb[:, a * EMBED : (a + 1) * EMBED], in_=wk_r[a]
                    )
                    nc.scalar.dma_start(
                        out=xt_sb[:, a * M : (a + 1) * M], in_=xt_r[a]
                    )
                for a in range(DC):
                    nc.gpsimd.dma_start(
                        out=wq_sb[:, a * EMBED : (a + 1) * EMBED], in_=wq_r[a]
                    )
                # K^T first so its AllGather overlaps the Q^T projection.
                for w_sb, d8, db, is_k in (
                    (wk_sb, kt8, ktb, True),
                    (wq_sb, qt8, qtb, False),
                ):
                    for j in range(HPR):  # row half
                        for b in range(DC):  # output dim chunk
                            ps = proj_ps.tile([P, NB], FP32, tag="proj_ps")
                            for a in range(DC):  # contraction chunk
                                nc.tensor.matmul(
                                    ps[:],
                                    lhsT=w_sb[:, a * EMBED + b * P : a * EMBED + (b + 1) * P],
                                    rhs=xt_sb[:, a * M + j * NB : a * M + (j + 1) * NB],
                                    start=(a == 0),
                                    stop=(a == DC - 1),
                                )
                            if b < F8C:
                                dst = d8[:, b, j * NB : (j + 1) * NB]
                            else:
                                dst = db[:, b - F8C, j * NB : (j + 1) * NB]
                            nc.vector.tensor_copy(out=dst, in_=ps[:])
                            if is_k:
                                if b < F8C:
                                    nc.sync.dma_start(out=ktd8[j][b], in_=dst)
                                else:
                                    nc.sync.dma_start(out=ktdb[j][b - F8C], in_=dst)
                        if is_k:
                            for td, gd in ((ktdb[j], gktb[j]), (ktd8[j], gkt8[j])):
                                nc.gpsimd.collective_compute(
                                    "AllGather",
                                    mybir.AluOpType.bypass,
                                    replica_groups=[list(range(NCORES))],
                                    ins=[td.opt()],
                                    outs=[gd.opt()],
                                )

            # ---- Phase B: streaming attention over key-block pairs
            with (
                tc.tile_pool(name="kv", bufs=4) as kvp,
                tc.tile_pool(name="pb", bufs=3) as pbp,
                tc.tile_pool(name="ps_s", bufs=3, space="PSUM") as ps_sp,
                tc.tile_pool(name="ps_u", bufs=2, space="PSUM") as ps_up,
                tc.tile_pool(name="ps_o", bufs=3, space="PSUM") as ps_op,
                tc.tile_pool(name="fin", bufs=2) as finp,
            ):
                def load_block(j, half):
                    """Fetch V and K tiles for block (j, half); return
                    (vtile, k8_slice(ap, c), kb_slice(bb, c))."""
                    vtile = kvp.tile([P, VC, EMBED], BF16, tag="vtile")
                    if j == 0:
                        for c in range(VC):
                            nc.sync.dma_start(
                                out=vtile[:, c, :], in_=xs_r[half * VC + c]
                            )

                        def k8s(ap, c, _h=half):
                            return kt8[
                                :, 2 * ap : 2 * ap + 2,
                                _h * NB + c * P : _h * NB + (c + 1) * P,
                            ]

                        def kbs(bb, c, _h=half):
                            return ktb[
                                :, bb, _h * NB + c * P : _h * NB + (c + 1) * P
                            ]

                    else:
                        src = (rank + j) % NCORES
                        for c in range(VC):
                            nc.gpsimd.dma_start(
                                out=vtile[:, c, :],
                                in_=xv_r[
                                    ds(src * (M // P) + half * VC + c, 1)
                                ].squeeze(0),
                            )
                        kt8_t = kvp.tile([P, F8C, NB], F8E4, tag="kt8_t")
                        ktb_t = kvp.tile([P, BFC, NB], BF16, tag="ktb_t")
                        for b in range(F8C):
                            nc.gpsimd.dma_start(
                                out=kt8_t[:, b, :],
                                in_=gkt8[half][ds(src * F8C + b, 1)].squeeze(0),
                            )
                        for b in range(BFC):
                            nc.gpsimd.dma_start(
                                out=ktb_t[:, b, :],
                                in_=gktb[half][ds(src * BFC + b, 1)].squeeze(0),
                            )

                        def k8s(ap, c, _t=kt8_t):
                            return _t[:, 2 * ap : 2 * ap + 2, c * P : (c + 1) * P]

                        def kbs(bb, c, _t=ktb_t):
                            return _t[:, bb, c * P : (c + 1) * P]

                    return vtile, k8s, kbs

                def scores_exp(k8s, kbs):
                    """Score chains + exp for one block; returns the
                    P^T tile (bf16, [P, VC, M])."""
                    pt_sb = pbp.tile([P, VC, M], BF16, tag="pt_sb")
                    for h in range(HPR):  # query column half
                        for c in range(VC):  # key chunk within block
                            ps_s = ps_sp.tile([P, NB], FP32, tag="ps_s")
                            for bb in range(BFC):
                                nc.tensor.matmul(
                                    ps_s[:],
                                    lhsT=kbs(bb, c),
                                    rhs=qtb[:, bb, h * NB : (h + 1) * NB],
                                    start=(bb == 0),
                                    stop=False,
                                )
                            for ap in range(F8C // 2):
                                nc.tensor.matmul(
                                    ps_s[:],
                                    lhsT=k8s(ap, c),
                                    rhs=qt8[:, 2 * ap : 2 * ap + 2, h * NB : (h + 1) * NB],
                                    start=False,
                                    stop=(ap == F8C // 2 - 1),
                                    perf_mode=DR,
                                )
                            nc.scalar.activation(
                                out=pt_sb[:, c, h * NB : (h + 1) * NB],
                                in_=ps_s[:],
                                func=EXP,
                            )
                    return pt_sb

                def sums_chain(pta, ptb, h, first):
                    # softmax partition-dim sums via ones-vector matmul,
                    # chained across both blocks of the pair.
                    ps_sum = ps_up.tile([P, NB], FP32, tag="ps_sum")
                    for bi, ptx in enumerate((pta, ptb)):
                        for c in range(VC):
                            nc.tensor.matmul(
                                ps_sum[:],
                                lhsT=ones_bf[:],
                                rhs=ptx[:, c, h * NB : (h + 1) * NB],
                                start=(bi == 0 and c == 0),
                                stop=(bi == 1 and c == VC - 1),
                            )
                    dsts = sums_acc[:, h * NB : (h + 1) * NB]
                    if first:
                        nc.vector.tensor_copy(out=dsts, in_=ps_sum[:])
                    else:
                        nc.vector.tensor_tensor(
                            out=dsts, in0=dsts, in1=ps_sum[:], op=ADD
                        )

                def pv_chains(va, vb, pta, ptb, h, first):
                    # PV^T: out^T[e, q] += V[k, e]^T @ P^T[k, q],
                    # chained across both blocks of the pair.
                    for e in range(DC):
                        ps_o = ps_op.tile([P, NB], FP32, tag="ps_o")
                        for vx, ptx in ((va, pta), (vb, ptb)):
                            for t in range(VC):
                                nc.tensor.matmul(
                                    ps_o[:],
                                    lhsT=vx[:, t, e * P : (e + 1) * P],
                                    rhs=ptx[:, t, h * NB : (h + 1) * NB],
                                    start=(vx is va and t == 0),
                                    stop=(vx is vb and t == VC - 1),
                                )
                        dst = out_acc[:, e * M + h * NB : e * M + (h + 1) * NB]
                        if first:
                            nc.vector.tensor_copy(out=dst, in_=ps_o[:])
                        else:
                            nc.vector.tensor_tensor(
                                out=dst, in0=dst, in1=ps_o[:], op=ADD
                            )

                def finalize(h):
                    # out_acc columns for query-half h are final: scale
                    # by 1/sums and ship out (overlaps remaining PV).
                    for e in range(DC):
                        outf = finp.tile([P, NB], BF16, tag="outf")
                        nc.vector.tensor_tensor(
                            out=outf[:],
                            in0=out_acc[:, e * M + h * NB : e * M + (h + 1) * NB],
                            in1=recip[:, h * NB : (h + 1) * NB],
                            op=MULT,
                        )
                        nc.sync.dma_start(
                            out=out_r[e][:, h * NB : (h + 1) * NB], in_=outf[:]
                        )

                for pi, (blk_a, blk_b) in enumerate(PAIRS):
                    first = pi == 0
                    last = pi == len(PAIRS) - 1
                    va, k8a, kba = load_block(*blk_a)
                    vb, k8b, kbb = load_block(*blk_b)
                    pta = scores_exp(k8a, kba)
                    ptb = scores_exp(k8b, kbb)
                    if not last:
                        for h in range(HPR):
                            sums_chain(pta, ptb, h, first)
                            pv_chains(va, vb, pta, ptb, h, first)
                    else:
                        # tail-hiding order: finish sums first so the
                        # reciprocal and the h=0 output scaling overlap
                        # the remaining PV matmul work.
                        for h in range(HPR):
                            sums_chain(pta, ptb, h, first)
                        nc.vector.reciprocal(out=recip[:], in_=sums_acc[:])
                        pv_chains(va, vb, pta, ptb, 0, first)
                        finalize(0)
                        pv_chains(va, vb, pta, ptb, 1, first)
                        finalize(1)

    nc.compile()
    return nc


_NC = None


def _get_nc():
    global _NC
    if _NC is None:
        _NC = _build()
    return _NC


def _run(x, rotation_params, entangle_params, **spmd_kwargs):
    import ml_dtypes

    x = np.ascontiguousarray(np.asarray(x, dtype=np.float32))
    sc = np.float32(1.0 / np.sqrt(np.sqrt(np.float32(EMBED))))
    wq = (np.asarray(rotation_params, dtype=np.float32).reshape(EMBED, EMBED) * sc).astype(
        ml_dtypes.bfloat16
    )
    wk = (np.asarray(entangle_params, dtype=np.float32).reshape(EMBED, EMBED) * sc).astype(
        ml_dtypes.bfloat16
    )
    x_bf = x.astype(ml_dtypes.bfloat16)
    xt_bf = np.ascontiguousarray(x_bf.T)
    in_maps = [
        {
            "xt_shard": np.ascontiguousarray(xt_bf[:, i * M : (i + 1) * M]),
            "x_shard": np.ascontiguousarray(x_bf[i * M : (i + 1) * M]),
            "x_full": x_bf,
            "wq": wq,
            "wk": wk,
        }
        for i in range(NCORES)
    ]
    res = bass_utils.run_bass_kernel_spmd(
        _get_nc(), in_maps, core_ids=list(range(NCORES)), **spmd_kwargs
    )
    out = np.concatenate(
        [np.asarray(res.results[i]["out"]).astype(np.float32).T for i in range(NCORES)],
        axis=0,
    )
    return np.ascontiguousarray(out), res


def kernel(x, rotation_params, entangle_params):
    out, _ = _run(x, rotation_params, entangle_params)
    return out


# revision 15
# speedup vs baseline: 1.0203x; 1.0195x over previous
"""Trainium2 Bass kernel for ClassicalSelfAttention.

  out = softmax((x @ Wq) @ (x @ Wk)^T / sqrt(D)) @ x      x: [8192, 1024] f32

Sharding (8 NeuronCores): rows of x are sharded across cores; each core
projects its own row-shard to Q^T and K^T, the K^T shards are AllGathered
across cores, and each core runs a streaming attention loop over 16
key-blocks of 512 keys: scores matmul -> fused exp on ScalarE -> PV matmul
accumulated in SBUF.  The scores matmul keeps K^T stationary and Q^T
moving, so PSUM holds scores TRANSPOSED ([key, query]); exp of that is
P^T directly, which is what the PV matmul consumes.

The kernel is power-limited (PE drops to ~2.0 GHz under sustained bf16
matmul load), so the main lever is fewer matmul instructions:
 - 6/8 of the 1024-dim score contraction runs in fp8e4 DoubleRow mode
   (2 contraction chunks per instruction): 8-MM chains -> 3 DR + 2 bf16.
   Sim-validated rel err ~1.83e-2 < 2e-2 (sim matched HW to ~1e-5 at
   the 4/8 setting).
 - Softmax denominators are a ones-vector matmul partition reduction
   chained across each block pair.
 - PV is computed transposed (out^T = V^T P^T with V chunks stationary):
   the final divide is an elementwise multiply by a replicated
   reciprocal row (no PE transposes); the host transposes the result.
 - Key blocks are processed in PAIRS: PV / sums accumulation chains span
   both blocks of a pair, halving PSUM->SBUF drain traffic on VectorE.
 - K^T AllGather is split into 4 collectives (fp8/bf16 x key-half) and
   remote blocks are processed half-0-first, so the gather is hidden
   behind own-block compute.
 - 1/sqrt(D) is folded as sqrt(1/32) into BOTH Wq and Wk on the host so
   Q,K land in fp8-friendly range (std ~0.11).
"""

import sys

import numpy as np

try:
    import concourse.bass as bass  # noqa: F401
except ImportError:  # pragma: no cover
    sys.path.insert(0, "/opt/trn_rl_repo")

import concourse.bacc as bacc
import concourse.mybir as mybir
import concourse.tile as tile
from concourse import bass_utils
from concourse.bass import ds

N_TOKENS = 8192
EMBED = 1024
NCORES = 8
M = N_TOKENS // NCORES  # rows per core (1024)
P = 128  # partitions
DC = EMBED // P  # contraction chunks (8)
NB = 512  # key-block width
VC = NB // P  # 128-wide key chunks per key block (4)
HPR = M // NB  # key-block halves per rank (2)

F8C = 6  # contraction chunks (of 8) done in fp8 DoubleRow
BFC = DC - F8C  # contraction chunks done in bf16

FP32 = mybir.dt.float32
BF16 = mybir.dt.bfloat16
F8E4 = mybir.dt.float8e4
EXP = mybir.ActivationFunctionType.Exp
ADD = mybir.AluOpType.add
MULT = mybir.AluOpType.mult
DR = mybir.MatmulPerfMode.DoubleRow

# key-block pairs (j, half): own pair first; then remote, half-0 first
# so the split AllGather pipelines; pair partners share PV/sums chains.
PAIRS = [((0, 0), (0, 1)), ((1, 0), (2, 0)), ((3, 0), (4, 0)),
         ((5, 0), (6, 0)), ((7, 0), (1, 1)), ((2, 1), (3, 1)),
         ((4, 1), (5, 1)), ((6, 1), (7, 1))]


def _build():
    nc = bacc.Bacc(
        "TRN2", target_bir_lowering=False, debug=False, num_devices=NCORES
    )
    xt_shard = nc.dram_tensor("xt_shard", [EMBED, M], BF16, kind="ExternalInput").ap()
    x_shard = nc.dram_tensor("x_shard", [M, EMBED], BF16, kind="ExternalInput").ap()
    x_full = nc.dram_tensor(
        "x_full", [N_TOKENS, EMBED], BF16, kind="ExternalInput"
    ).ap()
    wq_d = nc.dram_tensor("wq", [EMBED, EMBED], BF16, kind="ExternalInput").ap()
    wk_d = nc.dram_tensor("wk", [EMBED, EMBED], BF16, kind="ExternalInput").ap()
    # out^T in bf16: [EMBED, M]; the host transposes back and widens.
    out_d = nc.dram_tensor("out", [EMBED, M], BF16, kind="ExternalOutput").ap()

    wq_r = wq_d.rearrange("(a p) d -> a p d", p=P)  # [DC, P, EMBED]
    wk_r = wk_d.rearrange("(a p) d -> a p d", p=P)
    xt_r = xt_shard.rearrange("(a p) m -> a p m", p=P)  # [DC, P, M]
    xs_r = x_shard.rearrange("(t p) d -> t p d", p=P)  # [M//P, P, EMBED]
    xv_r = x_full.rearrange("(t p) d -> t p d", p=P)  # [64, P, EMBED]
    out_r = out_d.rearrange("(e p) m -> e p m", p=P)  # [DC, P, M]

    with tile.TileContext(nc) as tc:
        with (
            tc.tile_pool(name="persist", bufs=1) as pers,
            tc.tile_pool(name="persist_dram", bufs=1, space="DRAM") as pdram,
        ):
            # own Q^T / K^T, split by precision: fp8 chunks 0..F8C-1,
            # bf16 chunks F8C..7.  Layout [P, chunk, M].
            qt8 = pers.tile([P, F8C, M], F8E4)
            qtb = pers.tile([P, BFC, M], BF16)
            kt8 = pers.tile([P, F8C, M], F8E4)
            ktb = pers.tile([P, BFC, M], BF16)
            ones_bf = pers.tile([P, P], BF16)
            nc.vector.memset(ones_bf[:], 1.0)
            # fp32 PV^T accumulator: [p, e*M + m] (e = embed chunk)
            out_acc = pers.tile([P, DC * M], FP32)
            # softmax denominators, replicated across partitions: [p, m]
            sums_acc = pers.tile([P, M], FP32)
            recip = pers.tile([P, M], FP32)
            # K^T shard (AllGather inputs) and gathered K^T of all cores,
            # split by precision and key-half for collective pipelining.
            ktd8 = [pdram.tile([F8C, P, NB], F8E4, name=f"ktd8_{h}") for h in range(2)]
            ktdb = [pdram.tile([BFC, P, NB], BF16, name=f"ktdb_{h}") for h in range(2)]
            gkt8 = [
                pdram.tile([NCORES * F8C, P, NB], F8E4, addr_space="Shared",
                           name=f"gkt8_{h}")
                for h in range(2)
            ]
            gktb = [
                pdram.tile([NCORES * BFC, P, NB], BF16, addr_space="Shared",
                           name=f"gktb_{h}")
                for h in range(2)
            ]

            rank = nc.gpsimd.partition_id()

            # ---- Phase A: project K^T then Q^T; AllGather K^T (split)
            with (
                tc.tile_pool(name="proj", bufs=1) as proj,
                tc.tile_pool(name="proj_ps", bufs=4, space="PSUM") as proj_ps,
            ):
                wq_sb = proj.tile([P, DC * EMBED], BF16)
                wk_sb = proj.tile([P, DC * EMBED], BF16)
                xt_sb = proj.tile([P, DC * M], BF16)
                # wk/xt feed the first (K) projection - issue them first.
                for a in range(DC):
                    nc.sync.dma_start(
                        out=wk_sb[:, a * EMBED : (a + 1) * EMBED], in_=wk_r[a]
                    )
                    nc.sync.dma_start(
                        out=xt_sb[:, a * M : (a + 1) * M], in_=xt_r[a]
                    )
                for a in range(DC):
                    nc.sync.dma_start(
                        out=wq_sb[:, a * EMBED : (a + 1) * EMBED], in_=wq_r[a]
                    )
                # K^T first so its AllGather overlaps the Q^T projection.
                for w_sb, d8, db, is_k in (
                    (wk_sb, kt8, ktb, True),
                    (wq_sb, qt8, qtb, False),
                ):
                    for j in range(HPR):  # row half
                        for b in range(DC):  # output dim chunk
                            ps = proj_ps.tile([P, NB], FP32, tag="proj_ps")
                            for a in range(DC):  # contraction chunk
                                nc.tensor.matmul(
                                    ps[:],
                                    lhsT=w_sb[:, a * EMBED + b * P : a * EMBED + (b + 1) * P],
                                    rhs=xt_sb[:, a * M + j * NB : a * M + (j + 1) * NB],
                                    start=(a == 0),
                                    stop=(a == DC - 1),
                                )
                            if b < F8C:
                                dst = d8[:, b, j * NB : (j + 1) * NB]
                            else:
                                dst = db[:, b - F8C, j * NB : (j + 1) * NB]
                            nc.vector.tensor_copy(out=dst, in_=ps[:])
                            if is_k:
                                if b < F8C:
                                    nc.sync.dma_start(out=ktd8[j][b], in_=dst)
                                else:
                                    nc.sync.dma_start(out=ktdb[j][b - F8C], in_=dst)
                        if is_k:
                            for td, gd in ((ktdb[j], gktb[j]), (ktd8[j], gkt8[j])):
                                nc.gpsimd.collective_compute(
                                    "AllGather",
                                    mybir.AluOpType.bypass,
                                    replica_groups=[list(range(NCORES))],
                                    ins=[td.opt()],
                                    outs=[gd.opt()],
                                )

            # ---- Phase B: streaming attention over key-block pairs
            with (
                tc.tile_pool(name="kv", bufs=4) as kvp,
                tc.tile_pool(name="pb", bufs=3) as pbp,
                tc.tile_pool(name="ps_s", bufs=3, space="PSUM") as ps_sp,
                tc.tile_pool(name="ps_u", bufs=2, space="PSUM") as ps_up,
                tc.tile_pool(name="ps_o", bufs=3, space="PSUM") as ps_op,
                tc.tile_pool(name="fin", bufs=2) as finp,
            ):
                def load_block(j, half):
                    """Fetch V and K tiles for block (j, half); return
                    (vtile, k8_slice(ap, c), kb_slice(bb, c))."""
                    vtile = kvp.tile([P, VC, EMBED], BF16, tag="vtile")
                    if j == 0:
                        for c in range(VC):
                            nc.sync.dma_start(
                                out=vtile[:, c, :], in_=xs_r[half * VC + c]
                            )

                        def k8s(ap, c, _h=half):
                            return kt8[
                                :, 2 * ap : 2 * ap + 2,
                                _h * NB + c * P : _h * NB + (c + 1) * P,
                            ]

                        def kbs(bb, c, _h=half):
                            return ktb[
                                :, bb, _h * NB + c * P : _h * NB + (c + 1) * P
                            ]

                    else:
                        src = (rank + j) % NCORES
                        for c in range(VC):
                            nc.gpsimd.dma_start(
                                out=vtile[:, c, :],
                                in_=xv_r[
                                    ds(src * (M // P) + half * VC + c, 1)
                                ].squeeze(0),
                            )
                        kt8_t = kvp.tile([P, F8C, NB], F8E4, tag="kt8_t")
                        ktb_t = kvp.tile([P, BFC, NB], BF16, tag="ktb_t")
                        for b in range(F8C):
                            nc.gpsimd.dma_start(
                                out=kt8_t[:, b, :],
                                in_=gkt8[half][ds(src * F8C + b, 1)].squeeze(0),
                            )
                        for b in range(BFC):
                            nc.gpsimd.dma_start(
                                out=ktb_t[:, b, :],
                                in_=gktb[half][ds(src * BFC + b, 1)].squeeze(0),
                            )

                        def k8s(ap, c, _t=kt8_t):
                            return _t[:, 2 * ap : 2 * ap + 2, c * P : (c + 1) * P]

                        def kbs(bb, c, _t=ktb_t):
                            return _t[:, bb, c * P : (c + 1) * P]

                    return vtile, k8s, kbs

                def scores_exp(k8s, kbs):
                    """Score chains + exp for one block; returns the
                    P^T tile (bf16, [P, VC, M])."""
                    pt_sb = pbp.tile([P, VC, M], BF16, tag="pt_sb")
                    for h in range(HPR):  # query column half
                        for c in range(VC):  # key chunk within block
                            ps_s = ps_sp.tile([P, NB], FP32, tag="ps_s")
                            for bb in range(BFC):
                                nc.tensor.matmul(
                                    ps_s[:],
                                    lhsT=kbs(bb, c),
                                    rhs=qtb[:, bb, h * NB : (h + 1) * NB],
                                    start=(bb == 0),
                                    stop=False,
                                )
                            for ap in range(F8C // 2):
                                nc.tensor.matmul(
                                    ps_s[:],
                                    lhsT=k8s(ap, c),
                                    rhs=qt8[:, 2 * ap : 2 * ap + 2, h * NB : (h + 1) * NB],
                                    start=False,
                                    stop=(ap == F8C // 2 - 1),
                                    perf_mode=DR,
                                )
                            nc.scalar.activation(
                                out=pt_sb[:, c, h * NB : (h + 1) * NB],
                                in_=ps_s[:],
                                func=EXP,
                            )
                    return pt_sb

                def sums_chain(pta, ptb, h, first):
                    # softmax partition-dim sums via ones-vector matmul,
                    # chained across both blocks of the pair.
                    ps_sum = ps_up.tile([P, NB], FP32, tag="ps_sum")
                    for bi, ptx in enumerate((pta, ptb)):
                        for c in range(VC):
                            nc.tensor.matmul(
                                ps_sum[:],
                                lhsT=ones_bf[:],
                                rhs=ptx[:, c, h * NB : (h + 1) * NB],
                                start=(bi == 0 and c == 0),
                                stop=(bi == 1 and c == VC - 1),
                            )
                    dsts = sums_acc[:, h * NB : (h + 1) * NB]
                    if first:
                        nc.vector.tensor_copy(out=dsts, in_=ps_sum[:])
                    else:
                        nc.vector.tensor_tensor(
                            out=dsts, in0=dsts, in1=ps_sum[:], op=ADD
                        )

                def pv_chains(va, vb, pta, ptb, h, first):
                    # PV^T: out^T[e, q] += V[k, e]^T @ P^T[k, q],
                    # chained across both blocks of the pair.
                    for e in range(DC):
                        ps_o = ps_op.tile([P, NB], FP32, tag="ps_o")
                        for vx, ptx in ((va, pta), (vb, ptb)):
                            for t in range(VC):
                                nc.tensor.matmul(
                                    ps_o[:],
                                    lhsT=vx[:, t, e * P : (e + 1) * P],
                                    rhs=ptx[:, t, h * NB : (h + 1) * NB],
                                    start=(vx is va and t == 0),
                                    stop=(vx is vb and t == VC - 1),
                                )
                        dst = out_acc[:, e * M + h * NB : e * M + (h + 1) * NB]
                        if first:
                            nc.vector.tensor_copy(out=dst, in_=ps_o[:])
                        else:
                            nc.vector.tensor_tensor(
                                out=dst, in0=dst, in1=ps_o[:], op=ADD
                            )

                def finalize(h):
                    # out_acc columns for query-half h are final: scale
                    # by 1/sums and ship out (overlaps remaining PV).
                    for e in range(DC):
                        outf = finp.tile([P, NB], BF16, tag="outf")
                        nc.vector.tensor_tensor(
                            out=outf[:],
                            in0=out_acc[:, e * M + h * NB : e * M + (h + 1) * NB],
                            in1=recip[:, h * NB : (h + 1) * NB],
                            op=MULT,
                        )
                        nc.sync.dma_start(
                            out=out_r[e][:, h * NB : (h + 1) * NB], in_=outf[:]
                        )

                for pi, (blk_a, blk_b) in enumerate(PAIRS):
                    first = pi == 0
                    last = pi == len(PAIRS) - 1
                    va, k8a, kba = load_block(*blk_a)
                    vb, k8b, kbb = load_block(*blk_b)
                    pta = scores_exp(k8a, kba)
                    ptb = scores_exp(k8b, kbb)
                    if not last:
                        for h in range(HPR):
                            sums_chain(pta, ptb, h, first)
                            pv_chains(va, vb, pta, ptb, h, first)
                    else:
                        # tail-hiding order: finish sums first so the
                        # reciprocal and the h=0 output scaling overlap
                        # the remaining PV matmul work.
                        for h in range(HPR):
                            sums_chain(pta, ptb, h, first)
                        nc.vector.reciprocal(out=recip[:], in_=sums_acc[:])
                        pv_chains(va, vb, pta, ptb, 0, first)
                        finalize(0)
                        pv_chains(va, vb, pta, ptb, 1, first)
                        finalize(1)

    nc.compile()
    return nc


_NC = None


def _get_nc():
    global _NC
    if _NC is None:
        _NC = _build()
    return _NC


def _run(x, rotation_params, entangle_params, **spmd_kwargs):
    import ml_dtypes

    x = np.ascontiguousarray(np.asarray(x, dtype=np.float32))
    sc = np.float32(1.0 / np.sqrt(np.sqrt(np.float32(EMBED))))
    wq = (np.asarray(rotation_params, dtype=np.float32).reshape(EMBED, EMBED) * sc).astype(
        ml_dtypes.bfloat16
    )
    wk = (np.asarray(entangle_params, dtype=np.float32).reshape(EMBED, EMBED) * sc).astype(
        ml_dtypes.bfloat16
    )
    x_bf = x.astype(ml_dtypes.bfloat16)
    xt_bf = np.ascontiguousarray(x_bf.T)
    in_maps = [
        {
            "xt_shard": np.ascontiguousarray(xt_bf[:, i * M : (i + 1) * M]),
            "x_shard": np.ascontiguousarray(x_bf[i * M : (i + 1) * M]),
            "x_full": x_bf,
            "wq": wq,
            "wk": wk,
        }
        for i in range(NCORES)
    ]
    res = bass_utils.run_bass_kernel_spmd(
        _get_nc(), in_maps, core_ids=list(range(NCORES)), **spmd_kwargs
    )
    out = np.concatenate(
        [np.asarray(res.results[i]["out"]).astype(np.float32).T for i in range(NCORES)],
        axis=0,
    )
    return np.ascontiguousarray(out), res


def kernel(x, rotation_params, entangle_params):
    out, _ = _run(x, rotation_params, entangle_params)
    return out


# revision 16
# speedup vs baseline: 1.0285x; 1.0081x over previous
"""Trainium2 Bass kernel for ClassicalSelfAttention.

  out = softmax((x @ Wq) @ (x @ Wk)^T / sqrt(D)) @ x      x: [8192, 1024] f32

Sharding (8 NeuronCores): rows of x are sharded across cores; each core
projects its own row-shard to Q^T and K^T, the K^T shards are AllGathered
across cores, and each core runs a streaming attention loop over 16
key-blocks of 512 keys: scores matmul -> fused exp on ScalarE -> PV matmul
accumulated in SBUF.  The scores matmul keeps K^T stationary and Q^T
moving, so PSUM holds scores TRANSPOSED ([key, query]); exp of that is
P^T directly, which is what the PV matmul consumes.

The kernel is power-limited (PE drops to ~2.0 GHz under sustained bf16
matmul load), so the main lever is fewer matmul instructions:
 - 6/8 of the 1024-dim score contraction runs in fp8e4 DoubleRow mode
   (2 contraction chunks per instruction): 8-MM chains -> 3 DR + 2 bf16.
   Sim-validated rel err ~1.83e-2 < 2e-2 (sim matched HW to ~1e-5 at
   the 4/8 setting).
 - Softmax denominators are a ones-vector matmul partition reduction
   chained across each block pair.
 - PV is computed transposed (out^T = V^T P^T with V chunks stationary):
   the final divide is an elementwise multiply by a replicated
   reciprocal row (no PE transposes); the host transposes the result.
 - Key blocks are processed in PAIRS: PV / sums accumulation chains span
   both blocks of a pair, halving PSUM->SBUF drain traffic on VectorE.
 - K^T AllGather is split into 4 collectives (fp8/bf16 x key-half) and
   remote blocks are processed half-0-first, so the gather is hidden
   behind own-block compute.
 - 1/sqrt(D) is folded as sqrt(1/32) into BOTH Wq and Wk on the host so
   Q,K land in fp8-friendly range (std ~0.11).
"""

import sys

import numpy as np

try:
    import concourse.bass as bass  # noqa: F401
except ImportError:  # pragma: no cover
    sys.path.insert(0, "/opt/trn_rl_repo")

import concourse.bacc as bacc
import concourse.mybir as mybir
import concourse.tile as tile
from concourse import bass_utils
from concourse.bass import ds

N_TOKENS = 8192
EMBED = 1024
NCORES = 8
M = N_TOKENS // NCORES  # rows per core (1024)
P = 128  # partitions
DC = EMBED // P  # contraction chunks (8)
NB = 512  # key-block width
VC = NB // P  # 128-wide key chunks per key block (4)
HPR = M // NB  # key-block halves per rank (2)

F8C = 6  # contraction chunks (of 8) done in fp8 DoubleRow
BFC = DC - F8C  # contraction chunks done in bf16

FP32 = mybir.dt.float32
BF16 = mybir.dt.bfloat16
F8E4 = mybir.dt.float8e4
EXP = mybir.ActivationFunctionType.Exp
ADD = mybir.AluOpType.add
MULT = mybir.AluOpType.mult
DR = mybir.MatmulPerfMode.DoubleRow

# key-block pairs (j, half): own pair first; then remote, half-0 first
# so the split AllGather pipelines; pair partners share PV/sums chains.
PAIRS = [((0, 0), (0, 1)), ((1, 0), (2, 0)), ((3, 0), (4, 0)),
         ((5, 0), (6, 0)), ((7, 0), (1, 1)), ((2, 1), (3, 1)),
         ((4, 1), (5, 1)), ((6, 1), (7, 1))]


def _build():
    nc = bacc.Bacc(
        "TRN2", target_bir_lowering=False, debug=False, num_devices=NCORES
    )
    xt_shard = nc.dram_tensor("xt_shard", [EMBED, M], BF16, kind="ExternalInput").ap()
    x_shard = nc.dram_tensor("x_shard", [M, EMBED], BF16, kind="ExternalInput").ap()
    x_full = nc.dram_tensor(
        "x_full", [N_TOKENS, EMBED], BF16, kind="ExternalInput"
    ).ap()
    wq_d = nc.dram_tensor("wq", [EMBED, EMBED], BF16, kind="ExternalInput").ap()
    wk_d = nc.dram_tensor("wk", [EMBED, EMBED], BF16, kind="ExternalInput").ap()
    # out^T in bf16: [EMBED, M]; the host transposes back and widens.
    out_d = nc.dram_tensor("out", [EMBED, M], BF16, kind="ExternalOutput").ap()

    wq_r = wq_d.rearrange("(a p) d -> a p d", p=P)  # [DC, P, EMBED]
    wk_r = wk_d.rearrange("(a p) d -> a p d", p=P)
    xt_r = xt_shard.rearrange("(a p) m -> a p m", p=P)  # [DC, P, M]
    xs_r = x_shard.rearrange("(t p) d -> t p d", p=P)  # [M//P, P, EMBED]
    xv_r = x_full.rearrange("(t p) d -> t p d", p=P)  # [64, P, EMBED]
    out_r = out_d.rearrange("(e p) m -> e p m", p=P)  # [DC, P, M]

    with tile.TileContext(nc) as tc:
        with (
            tc.tile_pool(name="persist", bufs=1) as pers,
            tc.tile_pool(name="persist_dram", bufs=1, space="DRAM") as pdram,
        ):
            # own Q^T / K^T, split by precision: fp8 chunks 0..F8C-1,
            # bf16 chunks F8C..7.  Layout [P, chunk, M].
            qt8 = pers.tile([P, F8C, M], F8E4)
            qtb = pers.tile([P, BFC, M], BF16)
            kt8 = pers.tile([P, F8C, M], F8E4)
            ktb = pers.tile([P, BFC, M], BF16)
            ones8 = pers.tile([P, 2, P], F8E4)
            nc.vector.memset(ones8[:], 1.0)
            # fp32 PV^T accumulator: [p, e*M + m] (e = embed chunk)
            out_acc = pers.tile([P, DC * M], FP32)
            # softmax denominators, replicated across partitions: [p, m]
            sums_acc = pers.tile([P, M], FP32)
            recip = pers.tile([P, M], FP32)
            # K^T shard (AllGather inputs) and gathered K^T of all cores,
            # split by precision and key-half for collective pipelining.
            ktd8 = [pdram.tile([F8C, P, NB], F8E4, name=f"ktd8_{h}") for h in range(2)]
            ktdb = [pdram.tile([BFC, P, NB], BF16, name=f"ktdb_{h}") for h in range(2)]
            gkt8 = [
                pdram.tile([NCORES * F8C, P, NB], F8E4, addr_space="Shared",
                           name=f"gkt8_{h}")
                for h in range(2)
            ]
            gktb = [
                pdram.tile([NCORES * BFC, P, NB], BF16, addr_space="Shared",
                           name=f"gktb_{h}")
                for h in range(2)
            ]

            rank = nc.gpsimd.partition_id()

            # ---- Phase A: project K^T then Q^T; AllGather K^T (split)
            with (
                tc.tile_pool(name="proj", bufs=1) as proj,
                tc.tile_pool(name="proj_ps", bufs=4, space="PSUM") as proj_ps,
            ):
                wq_sb = proj.tile([P, DC * EMBED], BF16)
                wk_sb = proj.tile([P, DC * EMBED], BF16)
                xt_sb = proj.tile([P, DC * M], BF16)
                # wk/xt feed the first (K) projection - issue them first.
                for a in range(DC):
                    nc.sync.dma_start(
                        out=wk_sb[:, a * EMBED : (a + 1) * EMBED], in_=wk_r[a]
                    )
                    nc.sync.dma_start(
                        out=xt_sb[:, a * M : (a + 1) * M], in_=xt_r[a]
                    )
                for a in range(DC):
                    nc.sync.dma_start(
                        out=wq_sb[:, a * EMBED : (a + 1) * EMBED], in_=wq_r[a]
                    )
                # K^T first so its AllGather overlaps the Q^T projection.
                for w_sb, d8, db, is_k in (
                    (wk_sb, kt8, ktb, True),
                    (wq_sb, qt8, qtb, False),
                ):
                    for j in range(HPR):  # row half
                        for b in range(DC):  # output dim chunk
                            ps = proj_ps.tile([P, NB], FP32, tag="proj_ps")
                            for a in range(DC):  # contraction chunk
                                nc.tensor.matmul(
                                    ps[:],
                                    lhsT=w_sb[:, a * EMBED + b * P : a * EMBED + (b + 1) * P],
                                    rhs=xt_sb[:, a * M + j * NB : a * M + (j + 1) * NB],
                                    start=(a == 0),
                                    stop=(a == DC - 1),
                                )
                            if b < F8C:
                                dst = d8[:, b, j * NB : (j + 1) * NB]
                            else:
                                dst = db[:, b - F8C, j * NB : (j + 1) * NB]
                            nc.vector.tensor_copy(out=dst, in_=ps[:])
                            if is_k:
                                if b < F8C:
                                    nc.sync.dma_start(out=ktd8[j][b], in_=dst)
                                else:
                                    nc.sync.dma_start(out=ktdb[j][b - F8C], in_=dst)
                        if is_k:
                            for td, gd in ((ktdb[j], gktb[j]), (ktd8[j], gkt8[j])):
                                nc.gpsimd.collective_compute(
                                    "AllGather",
                                    mybir.AluOpType.bypass,
                                    replica_groups=[list(range(NCORES))],
                                    ins=[td.opt()],
                                    outs=[gd.opt()],
                                )

            # ---- Phase B: streaming attention over key-block pairs
            with (
                tc.tile_pool(name="kv", bufs=4) as kvp,
                tc.tile_pool(name="pb", bufs=3) as pbp,
                tc.tile_pool(name="ps_s", bufs=3, space="PSUM") as ps_sp,
                tc.tile_pool(name="ps_u", bufs=2, space="PSUM") as ps_up,
                tc.tile_pool(name="ps_o", bufs=3, space="PSUM") as ps_op,
                tc.tile_pool(name="fin", bufs=2) as finp,
            ):
                def load_block(j, half):
                    """Fetch V and K tiles for block (j, half); return
                    (vtile, k8_slice(ap, c), kb_slice(bb, c))."""
                    vtile = kvp.tile([P, VC, EMBED], BF16, tag="vtile")
                    if j == 0:
                        for c in range(VC):
                            nc.sync.dma_start(
                                out=vtile[:, c, :], in_=xs_r[half * VC + c]
                            )

                        def k8s(ap, c, _h=half):
                            return kt8[
                                :, 2 * ap : 2 * ap + 2,
                                _h * NB + c * P : _h * NB + (c + 1) * P,
                            ]

                        def kbs(bb, c, _h=half):
                            return ktb[
                                :, bb, _h * NB + c * P : _h * NB + (c + 1) * P
                            ]

                    else:
                        src = (rank + j) % NCORES
                        for c in range(VC):
                            nc.gpsimd.dma_start(
                                out=vtile[:, c, :],
                                in_=xv_r[
                                    ds(src * (M // P) + half * VC + c, 1)
                                ].squeeze(0),
                            )
                        kt8_t = kvp.tile([P, F8C, NB], F8E4, tag="kt8_t")
                        ktb_t = kvp.tile([P, BFC, NB], BF16, tag="ktb_t")
                        for b in range(F8C):
                            nc.gpsimd.dma_start(
                                out=kt8_t[:, b, :],
                                in_=gkt8[half][ds(src * F8C + b, 1)].squeeze(0),
                            )
                        for b in range(BFC):
                            nc.gpsimd.dma_start(
                                out=ktb_t[:, b, :],
                                in_=gktb[half][ds(src * BFC + b, 1)].squeeze(0),
                            )

                        def k8s(ap, c, _t=kt8_t):
                            return _t[:, 2 * ap : 2 * ap + 2, c * P : (c + 1) * P]

                        def kbs(bb, c, _t=ktb_t):
                            return _t[:, bb, c * P : (c + 1) * P]

                    return vtile, k8s, kbs

                def scores_exp(k8s, kbs):
                    """Score chains + exp for one block; returns the
                    P^T tile (bf16, [P, VC, M])."""
                    pt_sb = pbp.tile([P, VC, M], BF16, tag="pt_sb")
                    pt8 = pbp.tile([P, VC, M], F8E4, tag="pt8")
                    for h in range(HPR):  # query column half
                        for c in range(VC):  # key chunk within block
                            ps_s = ps_sp.tile([P, NB], FP32, tag="ps_s")
                            for bb in range(BFC):
                                nc.tensor.matmul(
                                    ps_s[:],
                                    lhsT=kbs(bb, c),
                                    rhs=qtb[:, bb, h * NB : (h + 1) * NB],
                                    start=(bb == 0),
                                    stop=False,
                                )
                            for ap in range(F8C // 2):
                                nc.tensor.matmul(
                                    ps_s[:],
                                    lhsT=k8s(ap, c),
                                    rhs=qt8[:, 2 * ap : 2 * ap + 2, h * NB : (h + 1) * NB],
                                    start=False,
                                    stop=(ap == F8C // 2 - 1),
                                    perf_mode=DR,
                                )
                            nc.scalar.activation(
                                out=pt_sb[:, c, h * NB : (h + 1) * NB],
                                in_=ps_s[:],
                                func=EXP,
                            )
                            nc.scalar.activation(
                                out=pt8[:, c, h * NB : (h + 1) * NB],
                                in_=ps_s[:],
                                func=EXP,
                            )
                    return pt_sb, pt8

                def sums_chain(pt8a, pt8b, h, first):
                    # softmax partition-dim sums via fp8 DoubleRow
                    # ones-vector matmul, chained across both blocks of
                    # the pair; emitted right after the score chains so
                    # the PE stays in DoubleRow mode (no mode-entry
                    # stall).  Denominator-side fp8 noise averages out.
                    ps_sum = ps_up.tile([P, NB], FP32, tag="ps_sum")
                    for bi, ptx in enumerate((pt8a, pt8b)):
                        for cp in range(VC // 2):
                            nc.tensor.matmul(
                                ps_sum[:],
                                lhsT=ones8[:],
                                rhs=ptx[:, 2 * cp : 2 * cp + 2, h * NB : (h + 1) * NB],
                                start=(bi == 0 and cp == 0),
                                stop=(bi == 1 and cp == VC // 2 - 1),
                                perf_mode=DR,
                            )
                    dsts = sums_acc[:, h * NB : (h + 1) * NB]
                    if first:
                        nc.vector.tensor_copy(out=dsts, in_=ps_sum[:])
                    else:
                        nc.vector.tensor_tensor(
                            out=dsts, in0=dsts, in1=ps_sum[:], op=ADD
                        )

                def pv_chains(va, vb, pta, ptb, h, first):
                    # PV^T: out^T[e, q] += V[k, e]^T @ P^T[k, q],
                    # chained across both blocks of the pair.
                    for e in range(DC):
                        ps_o = ps_op.tile([P, NB], FP32, tag="ps_o")
                        for vx, ptx in ((va, pta), (vb, ptb)):
                            for t in range(VC):
                                nc.tensor.matmul(
                                    ps_o[:],
                                    lhsT=vx[:, t, e * P : (e + 1) * P],
                                    rhs=ptx[:, t, h * NB : (h + 1) * NB],
                                    start=(vx is va and t == 0),
                                    stop=(vx is vb and t == VC - 1),
                                )
                        dst = out_acc[:, e * M + h * NB : e * M + (h + 1) * NB]
                        if first:
                            nc.vector.tensor_copy(out=dst, in_=ps_o[:])
                        else:
                            nc.vector.tensor_tensor(
                                out=dst, in0=dst, in1=ps_o[:], op=ADD
                            )

                def finalize(h):
                    # out_acc columns for query-half h are final: scale
                    # by 1/sums and ship out (overlaps remaining PV).
                    for e in range(DC):
                        outf = finp.tile([P, NB], BF16, tag="outf")
                        nc.vector.tensor_tensor(
                            out=outf[:],
                            in0=out_acc[:, e * M + h * NB : e * M + (h + 1) * NB],
                            in1=recip[:, h * NB : (h + 1) * NB],
                            op=MULT,
                        )
                        nc.sync.dma_start(
                            out=out_r[e][:, h * NB : (h + 1) * NB], in_=outf[:]
                        )

                for pi, (blk_a, blk_b) in enumerate(PAIRS):
                    first = pi == 0
                    last = pi == len(PAIRS) - 1
                    va, k8a, kba = load_block(*blk_a)
                    vb, k8b, kbb = load_block(*blk_b)
                    pta, pt8a = scores_exp(k8a, kba)
                    ptb, pt8b = scores_exp(k8b, kbb)
                    for h in range(HPR):
                        sums_chain(pt8a, pt8b, h, first)
                    if not last:
                        for h in range(HPR):
                            pv_chains(va, vb, pta, ptb, h, first)
                    else:
                        # tail-hiding order: sums done, so the
                        # reciprocal and the h=0 output scaling overlap
                        # the remaining PV matmul work.
                        nc.vector.reciprocal(out=recip[:], in_=sums_acc[:])
                        pv_chains(va, vb, pta, ptb, 0, first)
                        finalize(0)
                        pv_chains(va, vb, pta, ptb, 1, first)
                        finalize(1)

    nc.compile()
    return nc


_NC = None


def _get_nc():
    global _NC
    if _NC is None:
        _NC = _build()
    return _NC


def _run(x, rotation_params, entangle_params, **spmd_kwargs):
    import ml_dtypes

    x = np.ascontiguousarray(np.asarray(x, dtype=np.float32))
    sc = np.float32(1.0 / np.sqrt(np.sqrt(np.float32(EMBED))))
    wq = (np.asarray(rotation_params, dtype=np.float32).reshape(EMBED, EMBED) * sc).astype(
        ml_dtypes.bfloat16
    )
    wk = (np.asarray(entangle_params, dtype=np.float32).reshape(EMBED, EMBED) * sc).astype(
        ml_dtypes.bfloat16
    )
    x_bf = x.astype(ml_dtypes.bfloat16)
    xt_bf = np.ascontiguousarray(x_bf.T)
    in_maps = [
        {
            "xt_shard": np.ascontiguousarray(xt_bf[:, i * M : (i + 1) * M]),
            "x_shard": np.ascontiguousarray(x_bf[i * M : (i + 1) * M]),
            "x_full": x_bf,
            "wq": wq,
            "wk": wk,
        }
        for i in range(NCORES)
    ]
    res = bass_utils.run_bass_kernel_spmd(
        _get_nc(), in_maps, core_ids=list(range(NCORES)), **spmd_kwargs
    )
    out = np.concatenate(
        [np.asarray(res.results[i]["out"]).astype(np.float32).T for i in range(NCORES)],
        axis=0,
    )
    return np.ascontiguousarray(out), res


def kernel(x, rotation_params, entangle_params):
    out, _ = _run(x, rotation_params, entangle_params)
    return out


# revision 17
# speedup vs baseline: 1.0290x; 1.0005x over previous
"""Trainium2 Bass kernel for ClassicalSelfAttention.

  out = softmax((x @ Wq) @ (x @ Wk)^T / sqrt(D)) @ x      x: [8192, 1024] f32

Sharding (8 NeuronCores): rows of x are sharded across cores; each core
projects its own row-shard to Q^T and K^T, the K^T shards are AllGathered
across cores, and each core runs a streaming attention loop over 16
key-blocks of 512 keys: scores matmul -> fused exp on ScalarE -> PV matmul
accumulated in SBUF.  The scores matmul keeps K^T stationary and Q^T
moving, so PSUM holds scores TRANSPOSED ([key, query]); exp of that is
P^T directly, which is what the PV matmul consumes.

The kernel is power-limited (PE drops to ~2.0 GHz under sustained bf16
matmul load), so the main lever is fewer matmul instructions:
 - 6/8 of the 1024-dim score contraction runs in fp8e4 DoubleRow mode
   (2 contraction chunks per instruction): 8-MM chains -> 3 DR + 2 bf16.
   Sim-validated rel err ~1.83e-2 < 2e-2 (sim matched HW to ~1e-5 at
   the 4/8 setting).
 - Softmax denominators are a ones-vector matmul partition reduction
   chained across each block pair.
 - PV is computed transposed (out^T = V^T P^T with V chunks stationary):
   the final divide is an elementwise multiply by a replicated
   reciprocal row (no PE transposes); the host transposes the result.
 - Key blocks are processed in PAIRS: PV / sums accumulation chains span
   both blocks of a pair, halving PSUM->SBUF drain traffic on VectorE.
 - K^T AllGather is split into 4 collectives (fp8/bf16 x key-half) and
   remote blocks are processed half-0-first, so the gather is hidden
   behind own-block compute.
 - 1/sqrt(D) is folded as sqrt(1/32) into BOTH Wq and Wk on the host so
   Q,K land in fp8-friendly range (std ~0.11).
"""

import sys

import numpy as np

try:
    import concourse.bass as bass  # noqa: F401
except ImportError:  # pragma: no cover
    sys.path.insert(0, "/opt/trn_rl_repo")

import concourse.bacc as bacc
import concourse.mybir as mybir
import concourse.tile as tile
from concourse import bass_utils
from concourse.bass import ds

N_TOKENS = 8192
EMBED = 1024
NCORES = 8
M = N_TOKENS // NCORES  # rows per core (1024)
P = 128  # partitions
DC = EMBED // P  # contraction chunks (8)
NB = 512  # key-block width
VC = NB // P  # 128-wide key chunks per key block (4)
HPR = M // NB  # key-block halves per rank (2)

F8C = 6  # contraction chunks (of 8) done in fp8 DoubleRow
BFC = DC - F8C  # contraction chunks done in bf16

FP32 = mybir.dt.float32
BF16 = mybir.dt.bfloat16
F8E4 = mybir.dt.float8e4
EXP = mybir.ActivationFunctionType.Exp
ADD = mybir.AluOpType.add
MULT = mybir.AluOpType.mult
DR = mybir.MatmulPerfMode.DoubleRow

# key-block pairs (j, half): own pair first; then remote, half-0 first
# so the split AllGather pipelines; pair partners share PV/sums chains.
PAIRS = [((0, 0), (0, 1)), ((1, 0), (2, 0)), ((3, 0), (4, 0)),
         ((5, 0), (6, 0)), ((7, 0), (1, 1)), ((2, 1), (3, 1)),
         ((4, 1), (5, 1)), ((6, 1), (7, 1))]


def _build():
    nc = bacc.Bacc(
        "TRN2", target_bir_lowering=False, debug=False, num_devices=NCORES
    )
    xt_shard = nc.dram_tensor("xt_shard", [EMBED, M], BF16, kind="ExternalInput").ap()
    x_shard = nc.dram_tensor("x_shard", [M, EMBED], BF16, kind="ExternalInput").ap()
    x_full = nc.dram_tensor(
        "x_full", [N_TOKENS, EMBED], BF16, kind="ExternalInput"
    ).ap()
    wq_d = nc.dram_tensor("wq", [EMBED, EMBED], BF16, kind="ExternalInput").ap()
    wk_d = nc.dram_tensor("wk", [EMBED, EMBED], BF16, kind="ExternalInput").ap()
    # out^T in bf16: [EMBED, M]; the host transposes back and widens.
    out_d = nc.dram_tensor("out", [EMBED, M], BF16, kind="ExternalOutput").ap()

    wq_r = wq_d.rearrange("(a p) d -> a p d", p=P)  # [DC, P, EMBED]
    wk_r = wk_d.rearrange("(a p) d -> a p d", p=P)
    xt_r = xt_shard.rearrange("(a p) m -> a p m", p=P)  # [DC, P, M]
    xs_r = x_shard.rearrange("(t p) d -> t p d", p=P)  # [M//P, P, EMBED]
    xv_r = x_full.rearrange("(t p) d -> t p d", p=P)  # [64, P, EMBED]
    out_r = out_d.rearrange("(e p) m -> e p m", p=P)  # [DC, P, M]

    with tile.TileContext(nc) as tc:
        with (
            tc.tile_pool(name="persist", bufs=1) as pers,
            tc.tile_pool(name="persist_dram", bufs=1, space="DRAM") as pdram,
        ):
            # own Q^T / K^T, split by precision: fp8 chunks 0..F8C-1,
            # bf16 chunks F8C..7.  Layout [P, chunk, M].
            qt8 = pers.tile([P, F8C, M], F8E4)
            qtb = pers.tile([P, BFC, M], BF16)
            kt8 = pers.tile([P, F8C, M], F8E4)
            ktb = pers.tile([P, BFC, M], BF16)
            ones8 = pers.tile([P, 2, P], F8E4)
            nc.vector.memset(ones8[:], 1.0)
            # fp32 PV^T accumulator: [p, e*M + m] (e = embed chunk)
            out_acc = pers.tile([P, DC * M], FP32)
            # softmax denominators, replicated across partitions: [p, m]
            sums_acc = pers.tile([P, M], FP32)
            recip = pers.tile([P, M], FP32)
            # K^T shard (AllGather inputs) and gathered K^T of all cores,
            # split by precision and key-half for collective pipelining.
            ktd8 = [pdram.tile([F8C, P, NB], F8E4, name=f"ktd8_{h}") for h in range(2)]
            ktdb = [pdram.tile([BFC, P, NB], BF16, name=f"ktdb_{h}") for h in range(2)]
            gkt8 = [
                pdram.tile([NCORES * F8C, P, NB], F8E4, addr_space="Shared",
                           name=f"gkt8_{h}")
                for h in range(2)
            ]
            gktb = [
                pdram.tile([NCORES * BFC, P, NB], BF16, addr_space="Shared",
                           name=f"gktb_{h}")
                for h in range(2)
            ]

            rank = nc.gpsimd.partition_id()

            # ---- Phase A: project K^T then Q^T; AllGather K^T (split)
            with (
                tc.tile_pool(name="proj", bufs=1) as proj,
                tc.tile_pool(name="proj_ps", bufs=4, space="PSUM") as proj_ps,
            ):
                wq_sb = proj.tile([P, DC * EMBED], BF16)
                wk_sb = proj.tile([P, DC * EMBED], BF16)
                xt_sb = proj.tile([P, DC * M], BF16)
                # wk/xt feed the first (K) projection - issue them first.
                for a in range(DC):
                    nc.sync.dma_start(
                        out=wk_sb[:, a * EMBED : (a + 1) * EMBED], in_=wk_r[a]
                    )
                    nc.sync.dma_start(
                        out=xt_sb[:, a * M : (a + 1) * M], in_=xt_r[a]
                    )
                for a in range(DC):
                    nc.sync.dma_start(
                        out=wq_sb[:, a * EMBED : (a + 1) * EMBED], in_=wq_r[a]
                    )
                # K^T first so its AllGather overlaps the Q^T projection.
                for w_sb, d8, db, is_k in (
                    (wk_sb, kt8, ktb, True),
                    (wq_sb, qt8, qtb, False),
                ):
                    for j in range(HPR):  # row half
                        for b in range(DC):  # output dim chunk
                            ps = proj_ps.tile([P, NB], FP32, tag="proj_ps")
                            for a in range(DC):  # contraction chunk
                                nc.tensor.matmul(
                                    ps[:],
                                    lhsT=w_sb[:, a * EMBED + b * P : a * EMBED + (b + 1) * P],
                                    rhs=xt_sb[:, a * M + j * NB : a * M + (j + 1) * NB],
                                    start=(a == 0),
                                    stop=(a == DC - 1),
                                )
                            if b < F8C:
                                dst = d8[:, b, j * NB : (j + 1) * NB]
                            else:
                                dst = db[:, b - F8C, j * NB : (j + 1) * NB]
                            nc.vector.tensor_copy(out=dst, in_=ps[:])
                            if is_k:
                                if b < F8C:
                                    nc.sync.dma_start(out=ktd8[j][b], in_=dst)
                                else:
                                    nc.sync.dma_start(out=ktdb[j][b - F8C], in_=dst)
                        if is_k:
                            for td, gd in ((ktdb[j], gktb[j]), (ktd8[j], gkt8[j])):
                                nc.gpsimd.collective_compute(
                                    "AllGather",
                                    mybir.AluOpType.bypass,
                                    replica_groups=[list(range(NCORES))],
                                    ins=[td.opt()],
                                    outs=[gd.opt()],
                                )

            # ---- Phase B: streaming attention over key-block pairs
            with (
                tc.tile_pool(name="kv", bufs=5) as kvp,
                tc.tile_pool(name="pb", bufs=4) as pbp,
                tc.tile_pool(name="ps_s", bufs=3, space="PSUM") as ps_sp,
                tc.tile_pool(name="ps_u", bufs=2, space="PSUM") as ps_up,
                tc.tile_pool(name="ps_o", bufs=3, space="PSUM") as ps_op,
                tc.tile_pool(name="fin", bufs=2) as finp,
            ):
                def load_block(j, half):
                    """Fetch V and K tiles for block (j, half); return
                    (vtile, k8_slice(ap, c), kb_slice(bb, c))."""
                    vtile = kvp.tile([P, VC, EMBED], BF16, tag="vtile")
                    if j == 0:
                        for c in range(VC):
                            nc.sync.dma_start(
                                out=vtile[:, c, :], in_=xs_r[half * VC + c]
                            )

                        def k8s(ap, c, _h=half):
                            return kt8[
                                :, 2 * ap : 2 * ap + 2,
                                _h * NB + c * P : _h * NB + (c + 1) * P,
                            ]

                        def kbs(bb, c, _h=half):
                            return ktb[
                                :, bb, _h * NB + c * P : _h * NB + (c + 1) * P
                            ]

                    else:
                        src = (rank + j) % NCORES
                        for c in range(VC):
                            nc.gpsimd.dma_start(
                                out=vtile[:, c, :],
                                in_=xv_r[
                                    ds(src * (M // P) + half * VC + c, 1)
                                ].squeeze(0),
                            )
                        kt8_t = kvp.tile([P, F8C, NB], F8E4, tag="kt8_t")
                        ktb_t = kvp.tile([P, BFC, NB], BF16, tag="ktb_t")
                        for b in range(F8C):
                            nc.gpsimd.dma_start(
                                out=kt8_t[:, b, :],
                                in_=gkt8[half][ds(src * F8C + b, 1)].squeeze(0),
                            )
                        for b in range(BFC):
                            nc.gpsimd.dma_start(
                                out=ktb_t[:, b, :],
                                in_=gktb[half][ds(src * BFC + b, 1)].squeeze(0),
                            )

                        def k8s(ap, c, _t=kt8_t):
                            return _t[:, 2 * ap : 2 * ap + 2, c * P : (c + 1) * P]

                        def kbs(bb, c, _t=ktb_t):
                            return _t[:, bb, c * P : (c + 1) * P]

                    return vtile, k8s, kbs

                def scores_exp(k8s, kbs):
                    """Score chains + exp for one block; returns the
                    P^T tile (bf16, [P, VC, M])."""
                    pt_sb = pbp.tile([P, VC, M], BF16, tag="pt_sb")
                    pt8 = pbp.tile([P, VC, M], F8E4, tag="pt8")
                    for h in range(HPR):  # query column half
                        for c in range(VC):  # key chunk within block
                            ps_s = ps_sp.tile([P, NB], FP32, tag="ps_s")
                            for bb in range(BFC):
                                nc.tensor.matmul(
                                    ps_s[:],
                                    lhsT=kbs(bb, c),
                                    rhs=qtb[:, bb, h * NB : (h + 1) * NB],
                                    start=(bb == 0),
                                    stop=False,
                                )
                            for ap in range(F8C // 2):
                                nc.tensor.matmul(
                                    ps_s[:],
                                    lhsT=k8s(ap, c),
                                    rhs=qt8[:, 2 * ap : 2 * ap + 2, h * NB : (h + 1) * NB],
                                    start=False,
                                    stop=(ap == F8C // 2 - 1),
                                    perf_mode=DR,
                                )
                            nc.scalar.activation(
                                out=pt_sb[:, c, h * NB : (h + 1) * NB],
                                in_=ps_s[:],
                                func=EXP,
                            )
                            nc.scalar.activation(
                                out=pt8[:, c, h * NB : (h + 1) * NB],
                                in_=ps_s[:],
                                func=EXP,
                            )
                    return pt_sb, pt8

                def sums_chain(pt8a, pt8b, h, first):
                    # softmax partition-dim sums via fp8 DoubleRow
                    # ones-vector matmul, chained across both blocks of
                    # the pair; emitted right after the score chains so
                    # the PE stays in DoubleRow mode (no mode-entry
                    # stall).  Denominator-side fp8 noise averages out.
                    ps_sum = ps_up.tile([P, NB], FP32, tag="ps_sum")
                    for bi, ptx in enumerate((pt8a, pt8b)):
                        for cp in range(VC // 2):
                            nc.tensor.matmul(
                                ps_sum[:],
                                lhsT=ones8[:],
                                rhs=ptx[:, 2 * cp : 2 * cp + 2, h * NB : (h + 1) * NB],
                                start=(bi == 0 and cp == 0),
                                stop=(bi == 1 and cp == VC // 2 - 1),
                                perf_mode=DR,
                            )
                    dsts = sums_acc[:, h * NB : (h + 1) * NB]
                    if first:
                        nc.vector.tensor_copy(out=dsts, in_=ps_sum[:])
                    else:
                        nc.vector.tensor_tensor(
                            out=dsts, in0=dsts, in1=ps_sum[:], op=ADD
                        )

                def pv_chains(va, vb, pta, ptb, h, first):
                    # PV^T: out^T[e, q] += V[k, e]^T @ P^T[k, q],
                    # chained across both blocks of the pair.
                    for e in range(DC):
                        ps_o = ps_op.tile([P, NB], FP32, tag="ps_o")
                        for vx, ptx in ((va, pta), (vb, ptb)):
                            for t in range(VC):
                                nc.tensor.matmul(
                                    ps_o[:],
                                    lhsT=vx[:, t, e * P : (e + 1) * P],
                                    rhs=ptx[:, t, h * NB : (h + 1) * NB],
                                    start=(vx is va and t == 0),
                                    stop=(vx is vb and t == VC - 1),
                                )
                        dst = out_acc[:, e * M + h * NB : e * M + (h + 1) * NB]
                        if first:
                            nc.vector.tensor_copy(out=dst, in_=ps_o[:])
                        else:
                            nc.vector.tensor_tensor(
                                out=dst, in0=dst, in1=ps_o[:], op=ADD
                            )

                def finalize(h):
                    # out_acc columns for query-half h are final: scale
                    # by 1/sums and ship out (overlaps remaining PV).
                    for e in range(DC):
                        outf = finp.tile([P, NB], BF16, tag="outf")
                        nc.vector.tensor_tensor(
                            out=outf[:],
                            in0=out_acc[:, e * M + h * NB : e * M + (h + 1) * NB],
                            in1=recip[:, h * NB : (h + 1) * NB],
                            op=MULT,
                        )
                        nc.sync.dma_start(
                            out=out_r[e][:, h * NB : (h + 1) * NB], in_=outf[:]
                        )

                for pi, (blk_a, blk_b) in enumerate(PAIRS):
                    first = pi == 0
                    last = pi == len(PAIRS) - 1
                    va, k8a, kba = load_block(*blk_a)
                    vb, k8b, kbb = load_block(*blk_b)
                    pta, pt8a = scores_exp(k8a, kba)
                    ptb, pt8b = scores_exp(k8b, kbb)
                    for h in range(HPR):
                        sums_chain(pt8a, pt8b, h, first)
                    if not last:
                        for h in range(HPR):
                            pv_chains(va, vb, pta, ptb, h, first)
                    else:
                        # tail-hiding order: sums done, so the
                        # reciprocal and the h=0 output scaling overlap
                        # the remaining PV matmul work.
                        nc.vector.reciprocal(out=recip[:], in_=sums_acc[:])
                        pv_chains(va, vb, pta, ptb, 0, first)
                        finalize(0)
                        pv_chains(va, vb, pta, ptb, 1, first)
                        finalize(1)

    nc.compile()
    return nc


_NC = None


def _get_nc():
    global _NC
    if _NC is None:
        _NC = _build()
    return _NC


def _run(x, rotation_params, entangle_params, **spmd_kwargs):
    import ml_dtypes

    x = np.ascontiguousarray(np.asarray(x, dtype=np.float32))
    sc = np.float32(1.0 / np.sqrt(np.sqrt(np.float32(EMBED))))
    wq = (np.asarray(rotation_params, dtype=np.float32).reshape(EMBED, EMBED) * sc).astype(
        ml_dtypes.bfloat16
    )
    wk = (np.asarray(entangle_params, dtype=np.float32).reshape(EMBED, EMBED) * sc).astype(
        ml_dtypes.bfloat16
    )
    x_bf = x.astype(ml_dtypes.bfloat16)
    xt_bf = np.ascontiguousarray(x_bf.T)
    in_maps = [
        {
            "xt_shard": np.ascontiguousarray(xt_bf[:, i * M : (i + 1) * M]),
            "x_shard": np.ascontiguousarray(x_bf[i * M : (i + 1) * M]),
            "x_full": x_bf,
            "wq": wq,
            "wk": wk,
        }
        for i in range(NCORES)
    ]
    res = bass_utils.run_bass_kernel_spmd(
        _get_nc(), in_maps, core_ids=list(range(NCORES)), **spmd_kwargs
    )
    out = np.concatenate(
        [np.asarray(res.results[i]["out"]).astype(np.float32).T for i in range(NCORES)],
        axis=0,
    )
    return np.ascontiguousarray(out), res


def kernel(x, rotation_params, entangle_params):
    out, _ = _run(x, rotation_params, entangle_params)
    return out
